# revision 4
# baseline (speedup 1.0000x reference)
"""DigitCaps dynamic-routing kernel for 8 Trainium2 NeuronCores — v5.

Problem: x(32,16384,8) f32, W(10,16384,8,16) f32 -> v(32,10,16) f32
  u_hat[b,j,p,o] = sum_d x[b,p,d] W[j,p,d,o]   (never materialized)
  3 routing iterations (softmax over j, weighted sums over p).

Measured lineage: v1 596us, v2 452us, v3 565us, v4 497us (re-measured
401us warm). v4's trace: Vector pegged 100% through both routing
iterations; Scalar 87us of psum drains; 80 small wz DMAs/run; the
z-consume add-tree ran as 120 overhead-bound 128-512 elem adds.

v5 changes:
  * wz relayout [NTG,128,J,TG,128]: ONE 1.28MB DMA per (it,tg) instead
    of 10 small ones (sync-sequencer relief)
  * consume: psum z drains land in per-j slots of one big tmpJ
    [128,J,TG,D,B] tile; x-mul runs in-place per j (V/G alternating);
    the d-add-tree is 3 big ops folded over (j,t) (V/G split halves),
    L2 in-place, L3 writes bb directly (it0) — j-t order of tmpJ makes
    the bb target a clean strided view
  * softmax y: y = e * (x*rec) — the 1/sum is folded into x once per
    t-group (xr), killing the separate c=e*rec gpsimd mul
  * squash scatter: v^T is DMA'd straight into the d-diagonal slots of
    a pre-zeroed vblk (8 strided DMAs) — no vmask multiply
  * s-phase j-groups (0,8),(8,2): 256 matmuls/it instead of 384
  * warmup collective at t=0 absorbs the first-collective barrier
"""
import numpy as np
import ml_dtypes
from functools import lru_cache

import concourse.bacc as bacc
import concourse.mybir as mybir
from concourse import tile
from concourse.bass_utils import run_bass_kernel_spmd

F32 = mybir.dt.float32
BF16 = mybir.dt.bfloat16
AX = mybir.AxisListType
ALU = mybir.AluOpType
ACTF = mybir.ActivationFunctionType

B, J, P, D, O = 32, 10, 16384, 8, 16
NCORES = 8
PL = P // NCORES          # 2048
T = PL // 128             # 16 tiles of 128 p's
TG = 4                    # t-group size
NTG = T // TG             # 4
JO = J * O                # 160
JB = J * B                # 320
JT = J * TG               # 40
JGS = [(0, 8), (8, 2)]    # (j0, width) j-groups for s-phase

VMUL_GPS = {1, 3, 5, 7, 9}   # consume muls routed to gpsimd
VCOPY_DVE = {0}              # consume psum drains routed to vector


def _emit(nc, n_cores):
    xb = nc.dram_tensor("xb", [128, T, D, B], BF16, kind="ExternalInput")
    ws = nc.dram_tensor("ws", [128, T, D, J, O], BF16, kind="ExternalInput")
    wz = nc.dram_tensor("wz", [NTG, 128, J, TG, 128], BF16,
                        kind="ExternalInput")
    ones16 = nc.dram_tensor("ones16", [O, O], BF16, kind="ExternalInput")
    s3pT = nc.dram_tensor("s3pT", [O, J, B], F32, kind="ExternalOutput")

    with tile.TileContext(nc) as tc:
        with (
            tc.tile_pool(name="per", bufs=1) as per,
            tc.tile_pool(name="wu", bufs=1, space="DRAM") as wup,
            tc.tile_pool(name="ypool", bufs=8) as ypool,
            tc.tile_pool(name="sm", bufs=2) as sm,
            tc.tile_pool(name="tj", bufs=1) as tjp,
            tc.tile_pool(name="u1p", bufs=2) as u1p,
            tc.tile_pool(name="wzg", bufs=2) as wzp,
            tc.tile_pool(name="small", bufs=1) as small,
            tc.tile_pool(name="sps", bufs=1, space="PSUM") as sps,
            tc.tile_pool(name="zps", bufs=2, space="PSUM") as zps,
            tc.tile_pool(name="dram", bufs=2, space="DRAM") as dramp,
        ):
            # warmup collective: junk contents on purpose — no input deps,
            # issues immediately, absorbs the first-collective barrier
            # under the it0 compute + loads.
            wu_in = wup.tile([B, 16], F32)
            wu_out = wup.tile([B, 16], F32)
            nc.gpsimd.collective_compute(
                "AllReduce", ALU.add,
                replica_groups=[list(range(n_cores))],
                ins=[wu_in[:].opt()], outs=[wu_out[:].opt()],
            )

            # load order: what it0's s-phase needs first (x, ws); wz is
            # pulled per t-group inside the iterations.
            x_sb = per.tile([128, T, D, B], BF16)
            nc.sync.dma_start(x_sb[:], xb[:, :, :, :])
            ws_t = []
            for t in range(T):
                w = per.tile([128, D, J, O], BF16, name=f"ws{t}")
                nc.sync.dma_start(w[:], ws[:, t, :, :, :])
                ws_t.append(w)
            one_sb = per.tile([O, O], BF16)
            nc.sync.dma_start(one_sb[:], ones16[:, :])
            # z-phase rhs: vblk[(d,o), j, d', b] = v[b,j,o] iff d==d'.
            # Off-diagonal (d!=d') slots are zeroed once; every squash
            # rewrites only the diagonal slots via strided DMA.
            vblk = per.tile([128, J, D, B], BF16)
            nc.vector.memset(vblk[:], 0.0)
            bb = per.tile([128, T, J, B], F32)

            y_t = [None] * T

            def allreduce(src_dram):
                out = dramp.tile([O, J, B], F32)
                nc.gpsimd.collective_compute(
                    "AllReduce", ALU.add,
                    replica_groups=[list(range(n_cores))],
                    ins=[src_dram[:].opt()], outs=[out[:].opt()],
                )
                return out

            def squash_scatter(cc_out):
                """cc_out (DRAM [O,J,B] f32 summed s) -> vblk diagonal.

                sq[j,b] = sum_o s^2 via a ones-vector matmul over the 16
                o-partitions; the squash scale f stays on-chip; v^T is
                DMA-replicated into the D diagonal slots of vblk."""
                s_fT = small.tile([O, J, B], F32)
                nc.sync.dma_start(s_fT[:], cc_out[:, :, :])
                ssq = small.tile([O, JB], BF16)
                sfv = s_fT.rearrange("o j b -> o (j b)")
                nc.vector.tensor_mul(ssq[:], sfv, sfv)
                sq_ps = sps.tile([128, 512], F32, tag="s0ps", name="sq_ps")
                nc.tensor.matmul(sq_ps[0:O, 0:JB], one_sb[:], ssq[:],
                                 start=True, stop=True)
                sqv = small.tile([O, JB], F32)
                nc.vector.tensor_copy(sqv[:], sq_ps[0:O, 0:JB])
                r_ = small.tile([O, JB], F32)
                nc.scalar.activation(r_[:], sqv[:], ACTF.Sqrt)
                den = small.tile([O, JB], F32)
                nc.vector.scalar_tensor_tensor(
                    den[:], sqv[:], 1.0, r_[:], ALU.add, ALU.mult)
                rc = small.tile([O, JB], F32)
                nc.vector.reciprocal(rc[:], den[:])
                f_ = small.tile([O, JB], F32)
                nc.vector.tensor_mul(f_[:], sqv[:], rc[:])
                vT = small.tile([O, J, B], BF16)
                nc.vector.tensor_mul(
                    vT.rearrange("o j b -> o (j b)"), sfv, f_[:])
                for d in range(D):
                    nc.sync.dma_start(vblk[d * O:(d + 1) * O, :, d, :],
                                      vT[:])

            # ---------------- it0 s-phase: c == 0.1 ----------------
            s0_ps = sps.tile([128, 512], F32, tag="s0ps")
            for t in range(T):
                for d in range(D):
                    nc.tensor.matmul(
                        s0_ps[0:B, 0:JO],
                        x_sb[:, t, d, :],
                        ws_t[t][:, d, :, :],
                        start=(t == 0 and d == 0),
                        stop=(t == T - 1 and d == D - 1),
                    )
            s_sb = small.tile([B, JO], F32)
            nc.scalar.activation(s_sb[:], s0_ps[0:B, 0:JO], ACTF.Copy,
                                 scale=0.1)
            cc0 = dramp.tile([O, J, B], F32)
            for j in range(J):
                nc.sync.dma_start(
                    cc0[:, j, :].rearrange("o b -> b o"),
                    s_sb[:, j * O:(j + 1) * O])
            squash_scatter(allreduce(cc0))

            def emit_softmax_y(tg):
                """c = softmax_j(bb) for t-group tg; y(t) = e * (x*rec)."""
                t0 = tg * TG
                e_tg = sm.tile([128, TG, J, B], BF16)
                nc.scalar.activation(e_tg[:], bb[:, t0:t0 + TG, :, :],
                                     ACTF.Exp)
                # sum over j as a contiguous add tree (strided
                # tensor_reduce measured 2.3us vs ~1.1us for this)
                es1 = sm.tile([128, TG, 5, B], BF16)
                nc.vector.tensor_add(es1[:], e_tg[:, :, 0:5, :],
                                     e_tg[:, :, 5:10, :])
                es2 = sm.tile([128, TG, 2, B], BF16)
                nc.vector.tensor_add(es2[:], es1[:, :, 0:2, :],
                                     es1[:, :, 2:4, :])
                es3 = sm.tile([128, TG, B], BF16)
                nc.vector.tensor_add(es3[:], es2[:, :, 0, :],
                                     es2[:, :, 1, :])
                se = sm.tile([128, TG, B], F32)
                nc.vector.tensor_add(se[:], es3[:], es1[:, :, 4, :])
                rec = sm.tile([128, TG, B], BF16)
                with nc.allow_low_precision(
                        reason="1/sum feeds bf16 softmax muls"):
                    nc.vector.reciprocal(rec[:], se[:])
                # fold 1/sum into x once: y = e * xr hits the DVE
                # double-broadcast fast path (~0.56ns/elem)
                xr = sm.tile([128, TG, D, B], BF16)
                nc.vector.tensor_mul(
                    xr[:], x_sb[:, t0:t0 + TG, :, :],
                    rec[:, :, None, :].broadcast_to([128, TG, D, B]))
                for t4 in range(TG):
                    t = t0 + t4
                    y = ypool.tile([128, J, D, B], BF16)
                    nc.vector.tensor_mul(
                        y[:],
                        e_tg[:, t4, :, None, :].broadcast_to([128, J, D, B]),
                        xr[:, t4, None, :, :].broadcast_to([128, J, D, B]))
                    y_t[t] = y

            def emit_z_tg(it, tg):
                """z matmuls + x-weighted d-sum for t-group tg -> bb."""
                wzg = wzp.tile([128, J, TG, 128], BF16)
                nc.sync.dma_start(wzg[:], wz[tg, :, :, :, :])
                tmpJ = tjp.tile([128, J, TG, D, B], BF16)
                xv = x_sb[:, tg * TG:(tg + 1) * TG, :, :]
                for j in range(J):
                    z_ps = zps.tile([128, TG, 256], F32)
                    for t4 in range(TG):
                        nc.tensor.matmul(
                            z_ps[:, t4, :],
                            wzg[:, j, t4, :],
                            vblk[:, j, :, :].rearrange("p d b -> p (d b)"),
                            start=(t4 % 2 == 0), stop=(t4 % 2 == 1))
                    zv = z_ps.rearrange("p t db -> p (t db)")
                    dst = tmpJ[:, j].rearrange("p t d b -> p (t d b)")
                    if j in VCOPY_DVE:
                        nc.vector.tensor_copy(dst, zv)
                    else:
                        nc.scalar.copy(dst, zv)
                    # in-place x-weighting of the drained z slot
                    eng = nc.gpsimd if j in VMUL_GPS else nc.vector
                    eng.tensor_mul(tmpJ[:, j], tmpJ[:, j], xv)
                # batched d-add-tree over all (j,t) at once; L2 in-place
                tj = tmpJ.rearrange("p j t d b -> p (j t) d b")
                u1 = u1p.tile([128, JT, 4, B], BF16)
                h = JT // 2
                nc.vector.tensor_add(u1[:, 0:h], tj[:, 0:h, 0:4, :],
                                     tj[:, 0:h, 4:8, :])
                nc.gpsimd.tensor_add(u1[:, h:JT], tj[:, h:JT, 0:4, :],
                                     tj[:, h:JT, 4:8, :])
                nc.vector.tensor_add(u1[:, :, 0:2, :], u1[:, :, 0:2, :],
                                     u1[:, :, 2:4, :])
                bb_v = (bb[:, tg * TG:(tg + 1) * TG, :, :]
                        .rearrange("p t j b -> p j t b"))
                u1v = u1.rearrange("p (j t) f b -> p j t f b", j=J)
                if it == 0:
                    nc.vector.tensor_add(bb_v, u1v[:, :, :, 0, :],
                                         u1v[:, :, :, 1, :])
                else:
                    uv3 = u1p.tile([128, JT, B], BF16, name="uv3")
                    nc.gpsimd.tensor_add(uv3[:], u1[:, :, 0, :],
                                         u1[:, :, 1, :])
                    nc.vector.tensor_add(
                        bb_v, bb_v,
                        uv3.rearrange("p (j t) b -> p j t b", j=J))
                emit_softmax_y(tg)

            def emit_s_chunk(ps_jg, chunk):
                """s matmuls for 4 t's, j-group-contiguous (LDW pipelines).
                stationary = ws [128,(jw,o)], moving = y [128,(jw),(b)],
                out[(j,o),(j,b)] accumulated over all (t,d)."""
                for gi, (j0, jw) in enumerate(JGS):
                    for t in range(chunk * TG, (chunk + 1) * TG):
                        for d in range(D):
                            nc.tensor.matmul(
                                ps_jg[gi][0:jw * O, 0:jw * B],
                                ws_t[t][:, d, j0:j0 + jw, :],
                                y_t[t][:, j0:j0 + jw, d, :],
                                start=(t == 0 and d == 0),
                                stop=(t == T - 1 and d == D - 1),
                            )

            def drain_s(ps_jg, dst):
                """psum diag blocks [(j,o),(j,b)] -> dst[o, j, b] DRAM."""
                for gi, (j0, jw) in enumerate(JGS):
                    zsb = small.tile([128, 256], F32, name=f"zsb{gi}")
                    nc.vector.tensor_copy(
                        zsb[0:jw * O, 0:jw * B],
                        ps_jg[gi][0:jw * O, 0:jw * B])
                    for jl in range(jw):
                        nc.sync.dma_start(
                            dst[:, j0 + jl, :],
                            zsb[jl * O:(jl + 1) * O, jl * B:(jl + 1) * B])

            # ---------------- routing iterations ----------------
            for it in range(2):
                last = (it == 1)
                ps_jg = [sps.tile([128, jw * B], F32, name=f"spsj{gi}")
                         for gi, (j0, jw) in enumerate(JGS)]
                emit_z_tg(it, 0)
                emit_z_tg(it, 1)
                emit_z_tg(it, 2)
                emit_s_chunk(ps_jg, 0)
                emit_z_tg(it, 3)
                emit_s_chunk(ps_jg, 1)
                emit_s_chunk(ps_jg, 2)
                emit_s_chunk(ps_jg, 3)
                if last:
                    drain_s(ps_jg, s3pT)
                else:
                    cc_in = dramp.tile([O, J, B], F32)
                    drain_s(ps_jg, cc_in)
                    squash_scatter(allreduce(cc_in))
    return nc


@lru_cache(maxsize=2)
def _build(n_cores):
    nc = bacc.Bacc("TRN2", target_bir_lowering=False, debug=False,
                   num_devices=n_cores)
    _emit(nc, n_cores)
    nc.compile()
    return nc


def _prep_inputs(x, W):
    """Host-side shard + relayout. Returns list of per-core input dicts."""
    x = np.asarray(x, dtype=np.float32)
    W = np.asarray(W, dtype=np.float32)
    one = np.ones((O, O), np.float32).astype(ml_dtypes.bfloat16)
    in_maps = []
    for c in range(NCORES):
        xc = x[:, c * PL:(c + 1) * PL, :]              # (B, PL, D)
        Wc = W[:, c * PL:(c + 1) * PL, :, :]           # (J, PL, D, O)
        xr = np.ascontiguousarray(
            xc.reshape(B, T, 128, D).transpose(2, 1, 3, 0))        # [128,T,D,B]
        wsr = np.ascontiguousarray(
            Wc.reshape(J, T, 128, D, O).transpose(2, 1, 3, 0, 4))  # [128,T,D,J,O]
        wzr = np.ascontiguousarray(
            Wc.reshape(J, T, 128, D, O).transpose(0, 3, 4, 1, 2)   # j,d,o,t,p
            .reshape(J, 128, NTG, TG, 128)                         # j,(d,o),tg,t4,p
            .transpose(2, 1, 0, 3, 4))                             # [NTG,128,J,TG,128]
        in_maps.append({
            "xb": xr.astype(ml_dtypes.bfloat16),
            "ws": wsr.astype(ml_dtypes.bfloat16),
            "wz": wzr.astype(ml_dtypes.bfloat16),
            "ones16": one,
        })
    return in_maps


def _squash_np(s):
    sq = np.sum(s * s, axis=-1, keepdims=True)
    return s * (sq / ((1.0 + sq) * np.sqrt(sq)))


def kernel(x, W):
    nc = _build(NCORES)
    in_maps = _prep_inputs(x, W)
    res = run_bass_kernel_spmd(nc, in_maps, list(range(NCORES)))
    s3 = np.zeros((B, J, O), np.float64)
    for r in res.results:
        s3 += r["s3pT"].astype(np.float64).transpose(2, 1, 0)
    v = _squash_np(s3)
    return v.astype(np.float32)
